# revision 13
# baseline (speedup 1.0000x reference)
"""Channel-wise Linear on 8 TRN2 NeuronCores — v25.

y[b, c, :] = x[b, c, :] @ W[c].T + b[c]   (B=64, C=128, F=1024, fp32 ref)

Sharding: channels split across 8 cores (16 each), no cross-core comm.

v25 structure:
  - W ships as float8e3 (e3m4, 4 mantissa bits) scaled by 2^6; x carries
    the inverse scale in bf16 (exact power-of-2).  Host-sim rel err
    1.15e-2 vs the 2e-2 gate.  Halves W DMA to 16.8 MB/core: the DMA
    floor is ~59 us and the kernel is DMA-paced.
  - Channels processed in PAIRS via PE column-group tiling: channel c
    occupies output partitions 0-63, c+1 partitions 64-127 of the same
    PSUM banks; their matmuls stream concurrently in the two column
    halves of the 128x128 array, halving PE time (~30 us total).
  - Per-pair bias seed: one K=2 matmul per 512-col PSUM bank with a
    [2,128] 0/1 selector as the stationary operand (row k seeds column
    group k with bias row k).
  - W DMA in 0.5 MB halves (k-tiles 0-3 / 4-7), alternating the two
    HWDGE rings per channel in a pair; matmuls chase the halves so PE
    bursts spread across each pair window and HAM stays warm.  Endgame
    pair streams 0.25 MB quarters to shrink the tail.
  - x per channel (128 KB) just ahead of that channel's W on the same
    ring; biases on SWDGE; outputs split across both rings.
"""

import numpy as np
import ml_dtypes

import concourse.bass as bass
import concourse.bacc as bacc
import concourse.mybir as mybir
from concourse import tile
from concourse import bass_utils

B, C, F = 64, 128, 1024
NCORES = 8
CPC = C // NCORES          # channels per core
KT = F // 128              # contraction tiles per channel
F32 = mybir.dt.float32
BF16 = mybir.dt.bfloat16
F8 = mybir.dt.float8e3     # e3m4: 4 mantissa bits, 1 byte

# W is shipped as e3m4 scaled by 2^6 (sigma 1/32 -> 2; range to ~10.6 of
# 15.5 max, <13% denormal); x carries the inverse scale in bf16 (exact).
WSCALE = 64.0

WBUFS = 12                 # channel-sized W buffers in flight
WARMUP = 30                # real (K=128, N=512) warm-up matmuls

_CACHE = {}


def _build():
    if "nc" in _CACHE:
        return _CACHE["nc"]
    nc = bacc.Bacc(
        "TRN2",
        target_bir_lowering=False,
        debug=False,
        enable_asserts=True,
        num_devices=NCORES,
    )
    CH = KT * F            # per-channel W columns per partition
    wf = nc.dram_tensor("wf", [128, CPC * CH], F8, kind="ExternalInput").ap()
    xs = nc.dram_tensor("xs", [128, CPC * KT * B], BF16, kind="ExternalInput").ap()
    bs = nc.dram_tensor("bs", [CPC // 2, 2, F], BF16, kind="ExternalInput").ap()
    slt = nc.dram_tensor("slt", [2, 128], BF16, kind="ExternalInput").ap()
    yc = nc.dram_tensor("yc", [CPC // 2, 128, F], BF16, kind="ExternalOutput").ap()

    with tile.TileContext(nc) as tc:
        with (
            tc.tile_pool(name="w", bufs=WBUFS) as wpool,
            tc.tile_pool(name="x", bufs=1) as xpool,
            tc.tile_pool(name="bi", bufs=2) as bpool,
            tc.tile_pool(name="one", bufs=1) as onepool,
            tc.tile_pool(name="o", bufs=3) as opool,
            tc.tile_pool(name="ps", bufs=7, space=bass.MemorySpace.PSUM) as pspool,
        ):
            # [2,128] selector: row k is 1 on column group k, 0 elsewhere
            sel = onepool.tile([2, 128], BF16)
            nc.gpsimd.dma_start(sel[:], slt)
            junk = onepool.tile([128, 128], BF16, tag="junk")
            nc.gpsimd.memset(junk[:], 0.0)

            # persistent PSUM target for warm-up/bridge matmuls
            bps = pspool.tile([128, 512], F32, tag="bps", bufs=1)

            # PE warm-up: REAL full-array matmuls (HAM watches array cell
            # activity) covering the DMA head until the first W half lands.
            for _ in range(WARMUP):
                nc.tensor.matmul(
                    bps[:], junk[:], junk[:, 0:1].broadcast_to((128, 512)),
                    start=True, stop=True,
                )

            x_all = xpool.tile([128, CPC * KT * B], BF16)

            # all biases hoisted onto SWDGE up front so later y-out
            # waits on the same ring never delay a seed matmul
            b_ts = []
            for p in range(CPC // 2):
                b_t = bpool.tile([2, F], BF16, bufs=CPC // 2)
                nc.gpsimd.dma_start(b_t[:], bs[p])
                b_ts.append(b_t)

            for p in range(CPC // 2):
                c0, c1 = 2 * p, 2 * p + 1
                b_t = b_ts[p]

                x_pair = []
                w_pair = []
                for ci, c in enumerate((c0, c1)):
                    x_t = x_all[:, c * KT * B:(c + 1) * KT * B]
                    w_t = wpool.tile([128, CH], F8)
                    eng = nc.scalar if ci else nc.sync
                    eng.dma_start(x_t, xs[:, c * KT * B:(c + 1) * KT * B])
                    nch = 4 if p in (0, CPC // 2 - 1) else 2
                    cq = CH // nch
                    for q in range(nch):
                        eng.dma_start(
                            w_t[:, q * cq:(q + 1) * cq],
                            wf[:, c * CH + q * cq:c * CH + (q + 1) * cq],
                        )
                    x_pair.append(x_t)
                    w_pair.append(w_t)

                ps0 = pspool.tile([128, 512], F32, tag="ps")
                ps1 = pspool.tile([128, 512], F32, tag="ps")
                # bias seed: column group k gets bias row k (K=2 matmul)
                nc.tensor.matmul(
                    ps0[:], sel[:], b_t[:, 0:512],
                    start=True, stop=False, skip_group_check=True,
                )
                nc.tensor.matmul(
                    ps1[:], sel[:], b_t[:, 512:F],
                    start=True, stop=False, skip_group_check=True,
                )
                for kt in range(KT):
                    last = kt == KT - 1
                    for ci in range(2):
                        xo = 64 * ci
                        lhsT = x_pair[ci][:, kt * B:(kt + 1) * B]
                        wk = w_pair[ci][:, kt * F:(kt + 1) * F]
                        nc.tensor.matmul(
                            ps0[xo:xo + 64, :], lhsT, wk[:, 0:512],
                            start=False, stop=last, skip_group_check=True,
                        )
                        nc.tensor.matmul(
                            ps1[xo:xo + 64, :], lhsT, wk[:, 512:F],
                            start=False, stop=last, skip_group_check=True,
                        )

                # bridge matmuls: reset the HAM MID window during the
                # DMA-paced idle until the next pair's W halves land
                for _ in range(2):
                    nc.tensor.matmul(
                        bps[:], junk[:], junk[:, 0:1].broadcast_to((128, 512)),
                        start=True, stop=True,
                    )

                o_t = opool.tile([128, F], BF16)
                nc.vector.tensor_copy(o_t[:, 0:512], ps0[:])
                nc.vector.tensor_copy(o_t[:, 512:F], ps1[:])
                # rows split the pair: 0-63 = channel c0, 64-127 = c1.
                # y rides SWDGE so its CAST-wait never blocks the pure
                # x+W streams on the HWDGE rings
                nc.gpsimd.dma_start(yc[p], o_t[:])

    nc.compile()
    _CACHE["nc"] = nc
    return nc


def shard_inputs(x, W, b):
    ndt = ml_dtypes.bfloat16
    wdt = ml_dtypes.float8_e3m4
    in_maps = []
    for core in range(NCORES):
        cs, ce = core * CPC, (core + 1) * CPC
        # wf[p, c*KT*F + kt*F + g] = W[cs+c][g][kt*128 + p] * WSCALE
        wt = (W[cs:ce] * WSCALE).astype(wdt).transpose(0, 2, 1)  # [CPC, f, g]
        wf = np.ascontiguousarray(
            wt.reshape(CPC, KT, 128, F).transpose(2, 0, 1, 3)
        ).reshape(128, CPC * KT * F)
        xt = (x[:, cs:ce, :] * (1.0 / WSCALE)).astype(ndt).transpose(1, 2, 0)
        xsh = np.ascontiguousarray(
            xt.reshape(CPC, KT, 128, B).transpose(2, 0, 1, 3)
        ).reshape(128, CPC * KT * B)
        bsh = np.ascontiguousarray(b[cs:ce].astype(ndt).reshape(CPC // 2, 2, F))
        slt = np.zeros((2, 128), dtype=ndt)
        slt[0, 0:64] = 1
        slt[1, 64:128] = 1
        in_maps.append({"wf": wf, "xs": xsh, "bs": bsh, "slt": slt})
    return in_maps


def gather_output(results):
    yc = np.stack([results[core]["yc"] for core in range(NCORES)])
    # [8, CPC//2, 128, F]: rows split the channel pair; channel = 2q + i
    y = yc.reshape(NCORES, CPC // 2, 2, B, F)
    y = y.reshape(C, B, F)                              # [core*q*i] = channel
    return np.ascontiguousarray(y.transpose(1, 0, 2).astype(np.float32))


def kernel(x, W, b):
    x = np.asarray(x)
    W = np.asarray(W)
    b = np.asarray(b)
    nc = _build()
    in_maps = shard_inputs(x, W, b)
    res = bass_utils.run_bass_kernel_spmd(nc, in_maps, core_ids=list(range(NCORES)))
    return gather_output(res.results)
